# revision 4
# baseline (speedup 1.0000x reference)
"""Causal self-attention (B=4, S=2048, D=1024, H=16) on 8 Trainium2 NeuronCores.

Sharding: core c handles batch c//2 and heads (c%2)*8 .. (c%2)*8+8
(tensor-parallel over heads x data-parallel over batch). Each core:
  qkv-proj for its 1536 W_attn columns, attention for its 8 heads, and a
  row-parallel output projection producing a partial [2048, 1024]. The host
  sums the two partials per batch and adds the bias constant
  (b_v @ W_proj + b_proj -- valid because softmax rows sum to 1, so the
  v-bias passes through attention unchanged).

Device layouts: scores are computed transposed ([k-tokens on partitions,
q on free]) so the attention output lands as y^T [head-dims, tokens], which
feeds the output projection directly without any transposes. The softmax
denominator rides along as a 65th column of ones appended to v. Causal
masking multiplies the 4 distinct diagonal-block patterns post-exp; fully
masked blocks are skipped entirely.
"""

from contextlib import ExitStack

import numpy as np

import concourse.mybir as mybir
import concourse.tile as tile
from concourse import bacc
from concourse.bass import ts
from concourse.bass_utils import run_bass_kernel_spmd

B, S, D, H, HD = 4, 2048, 1024, 16, 64
P = 128
NHL = 8        # heads per core
DL = NHL * HD  # 512 local dims per of q/k/v
NT = S // 512  # 4 token tiles
NKB = S // P   # 16 k blocks
NQB = S // 512 # 4 q blocks
F32 = mybir.dt.float32
F32R = mybir.dt.float32r
AF = mybir.ActivationFunctionType
MULT = mybir.AluOpType.mult

_CACHE = {}


def build_program():
    if "nc" in _CACHE:
        return _CACHE["nc"]
    nc = bacc.Bacc("TRN2", target_bir_lowering=False, debug=False)
    xT = nc.dram_tensor("xT", [D, S], F32R, kind="ExternalInput").ap()
    wqk = nc.dram_tensor("wqk", [D, 2 * DL], F32R, kind="ExternalInput").ap()
    wv = nc.dram_tensor("wv", [D, DL], F32R, kind="ExternalInput").ap()
    bqk = nc.dram_tensor("bqk", [P, 8], F32, kind="ExternalInput").ap()
    wp = nc.dram_tensor("wp", [DL, D], F32R, kind="ExternalInput").ap()
    outT = nc.dram_tensor("outT", [D, S], F32, kind="ExternalOutput").ap()

    xT_r = xT.rearrange("(c p) t -> p c t", p=P)      # [128, 8, 2048]
    outT_r = outT.rearrange("(c p) t -> p c t", p=P)  # [128, 8, 2048]

    with tile.TileContext(nc) as tc:
        with ExitStack() as stack:
            const = stack.enter_context(tc.tile_pool(name="const", bufs=1))
            poolAB = stack.enter_context(tc.tile_pool(name="poolAB", bufs=1))

            # ---- constants
            masks = []
            for t in range(4):
                m = const.tile([P, 512], F32, tag=f"mask{t}")
                nc.gpsimd.memset(m[:], 1.0)
                # keep (p=1) iff q_j >= 128*t + k_i within the diagonal block
                nc.gpsimd.affine_select(
                    out=m[:],
                    in_=m[:],
                    compare_op=mybir.AluOpType.is_ge,
                    fill=0.0,
                    base=-(P * t),
                    pattern=[[1, 512]],
                    channel_multiplier=-1,
                )
                masks.append(m)
            bqk_sb = const.tile([P, 8], F32, tag="bqk")
            nc.sync.dma_start(bqk_sb[:], bqk)
            ones_row_f = const.tile([1, HD], F32, tag="ones_f")
            nc.gpsimd.memset(ones_row_f[:], 1.0)
            ones_row = const.tile([1, HD], F32R, tag="ones_r")
            nc.vector.tensor_copy(ones_row[:], ones_row_f[:])
            ones_v = const.tile([P, NKB, NHL], F32, tag="ones_v")
            nc.gpsimd.memset(ones_v[:], 1.0)

            # ---- persistent intermediates (phases A+B)
            qkT = poolAB.tile([P, 8, S], F32R, tag="qkT")  # chunks: q0..q3 k0..k3
            v_sb = poolAB.tile([P, NKB, NHL, HD + 1], F32R, tag="v")
            nc.vector.tensor_copy(v_sb[:, :, :, HD], ones_v[:])

            # =============== phase A: qkv projections =================
            with ExitStack() as stA:
                poolA = stA.enter_context(tc.tile_pool(name="poolA", bufs=1))
                xpool = stA.enter_context(tc.tile_pool(name="xpool", bufs=2))
                psA = stA.enter_context(
                    tc.tile_pool(name="psA", bufs=4, space="PSUM")
                )
                wqk_sb = poolA.tile([P, 8, 2 * DL], F32R, tag="wqk")
                nc.sync.dma_start(wqk_sb[:], wqk.rearrange("(c p) n -> p c n", p=P))
                wv_sb = poolA.tile([P, 8, DL], F32R, tag="wv")
                nc.sync.dma_start(wv_sb[:], wv.rearrange("(c p) n -> p c n", p=P))

                for tt in range(NT):
                    xt = xpool.tile([P, 8, 512], F32R, tag="xt")
                    nc.sync.dma_start(xt[:], xT_r[:, :, ts(tt, 512)])
                    # q^T / k^T chunks [128 dims, 512 tokens]
                    for j in range(8):
                        pq = psA.tile([P, 512], F32, tag="psA")
                        for kc in range(8):
                            nc.tensor.matmul(
                                pq[:],
                                wqk_sb[:, kc, ts(j, P)],
                                xt[:, kc, :],
                                start=(kc == 0),
                                stop=(kc == 7),
                            )
                        nc.vector.tensor_scalar_add(
                            qkT[:, j, ts(tt, 512)], pq[:], bqk_sb[:, j : j + 1]
                        )
                    # v token-major [128 tokens, 512 vdims]
                    for u in range(4):
                        pv = psA.tile([P, 512], F32, tag="psA")
                        for kc in range(8):
                            nc.tensor.matmul(
                                pv[:],
                                xt[:, kc, ts(u, P)],
                                wv_sb[:, kc, :],
                                start=(kc == 0),
                                stop=(kc == 7),
                            )
                        nc.vector.tensor_copy(
                            v_sb[:, tt * 4 + u, :, 0:HD],
                            pv[:].rearrange("p (h d) -> p h d", d=HD),
                        )

            # =============== phases B + C pools =================
            poolBC = stack.enter_context(tc.tile_pool(name="poolBC", bufs=1))
            stB = stack.enter_context(ExitStack())
            workB = stB.enter_context(tc.tile_pool(name="workB", bufs=3))
            psB = stB.enter_context(tc.tile_pool(name="psB", bufs=1, space="PSUM"))
            psScr = stB.enter_context(
                tc.tile_pool(name="psScr", bufs=2, space="PSUM")
            )

            yT = poolBC.tile([P, 4, S], F32R, tag="yT")
            wp_sb = poolBC.tile([P, 4, D], F32R, tag="wp")
            nc.sync.dma_start(wp_sb[:], wp.rearrange("(c p) n -> p c n", p=P))

            # =============== phase B: attention =================
            for hp in range(4):
                heads = (2 * hp, 2 * hp + 1)
                for qb in range(NQB):
                    nkb = 4 * qb + 4
                    pos = {}
                    for h in heads:
                        pos[h] = psB.tile(
                            [HD + 1, 512], F32, tag=f"po{h % 2}", name=f"po{h % 2}"
                        )
                    for kb in range(nkb):
                        for h in heads:
                            off = (h % 2) * HD
                            j = h // 2
                            pscr = psScr.tile([P, 512], F32, tag=f"pscr{h % 2}")
                            nc.tensor.matmul(
                                pscr[:],
                                qkT[off : off + HD, 4 + j, ts(kb, P)],
                                qkT[off : off + HD, j, ts(qb, 512)],
                                start=True,
                                stop=True,
                            )
                            p_sb = workB.tile([P, 512], F32R, tag=f"p{h % 2}")
                            nc.scalar.activation(
                                p_sb[:], pscr[:], AF.Exp, scale=0.125
                            )
                            if kb >= 4 * qb:
                                nc.vector.tensor_tensor(
                                    p_sb[:], p_sb[:], masks[kb - 4 * qb][:], MULT
                                )
                            nc.tensor.matmul(
                                pos[h][:],
                                v_sb[:, kb, h, :],
                                p_sb[:],
                                start=(kb == 0),
                                stop=(kb == nkb - 1),
                            )
                    for h in heads:
                        off = (h % 2) * HD
                        j = h // 2
                        po = pos[h]
                        rec = workB.tile([1, 512], F32R, tag="rec")
                        with nc.allow_low_precision(reason="f32r is fp32 storage"):
                            nc.vector.reciprocal(rec[:], po[HD : HD + 1, :])
                        pr = psScr.tile([HD, 512], F32, tag="pr")
                        nc.tensor.matmul(
                            pr[:], ones_row[:], rec[:], start=True, stop=True
                        )
                        onum = workB.tile([HD, 512], F32, tag="onum")
                        nc.vector.tensor_copy(onum[:], po[0:HD, :])
                        nc.vector.tensor_tensor(
                            yT[off : off + HD, j, ts(qb, 512)], onum[:], pr[:], MULT
                        )

            # =============== phase C: output projection =================
            stB.close()
            with ExitStack() as stC:
                workC = stC.enter_context(tc.tile_pool(name="workC", bufs=3))
                psC = stC.enter_context(
                    tc.tile_pool(name="psC", bufs=4, space="PSUM")
                )
                for oc in range(8):
                    for tt in range(NT):
                        pp = psC.tile([P, 512], F32, tag="pp")
                        for dc in range(4):
                            nc.tensor.matmul(
                                pp[:],
                                wp_sb[:, dc, ts(oc, P)],
                                yT[:, dc, ts(tt, 512)],
                                start=(dc == 0),
                                stop=(dc == 3),
                            )
                        ob = workC.tile([P, 512], F32, tag="ob")
                        nc.vector.tensor_copy(ob[:], pp[:])
                        nc.sync.dma_start(outT_r[:, oc, ts(tt, 512)], ob[:])

    nc.compile()
    _CACHE["nc"] = nc
    return nc


def make_in_maps(x, W_attn, b_attn, W_proj):
    x = np.asarray(x, dtype=np.float32)
    W_attn = np.asarray(W_attn, dtype=np.float32)
    b_attn = np.asarray(b_attn, dtype=np.float32)
    W_proj = np.asarray(W_proj, dtype=np.float32)
    in_maps = []
    for c in range(8):
        b, g = divmod(c, 2)
        q_sl = slice(g * DL, (g + 1) * DL)
        k_sl = slice(D + g * DL, D + (g + 1) * DL)
        v_sl = slice(2 * D + g * DL, 2 * D + (g + 1) * DL)
        in_maps.append(
            {
                "xT": np.ascontiguousarray(x[b].T),
                "wqk": np.ascontiguousarray(
                    np.concatenate([W_attn[:, q_sl], W_attn[:, k_sl]], axis=1)
                ),
                "wv": np.ascontiguousarray(W_attn[:, v_sl]),
                "bqk": np.ascontiguousarray(
                    np.concatenate([b_attn[q_sl], b_attn[k_sl]]).reshape(8, P).T
                ),
                "wp": np.ascontiguousarray(W_proj[g * DL : (g + 1) * DL, :]),
            }
        )
    return in_maps


def combine(results, b_attn, W_proj, b_proj):
    const_row = b_attn[2 * D :].astype(np.float32) @ W_proj + b_proj
    out = np.empty((B, S, D), dtype=np.float32)
    for b in range(B):
        acc = results[2 * b]["outT"] + results[2 * b + 1]["outT"]  # [D, S]
        out[b] = acc.T + const_row[None, :]
    return out


def kernel(x, W_attn, b_attn, W_proj, b_proj):
    nc = build_program()
    in_maps = make_in_maps(x, W_attn, b_attn, W_proj)
    res = run_bass_kernel_spmd(nc, in_maps, core_ids=list(range(8)))
    return combine(
        res.results,
        np.asarray(b_attn, np.float32),
        np.asarray(W_proj, np.float32),
        np.asarray(b_proj, np.float32),
    )


# revision 29
# speedup vs baseline: 1.0028x; 1.0028x over previous
"""Causal self-attention (B=4, S=2048, D=1024, H=16) on 8 Trainium2 NeuronCores.

Sharding: core c handles batch c//2 and heads (c%2)*8 .. (c%2)*8+8
(tensor-parallel over heads x data-parallel over batch). Each core:
  qkv-proj for its 1536 W_attn columns, attention for its 8 heads, and a
  row-parallel output projection producing a partial [2048, 1024]. The host
  sums the two partials per batch and adds the bias constant
  (b_v @ W_proj + b_proj -- valid because softmax rows sum to 1, so the
  v-bias passes through attention unchanged).

Device layouts: scores are computed transposed ([k-tokens on partitions,
q on free]) so the attention output lands as y^T [head-dims, tokens], which
feeds the output projection directly without any transposes. The softmax
denominator rides along as a 65th column of ones appended to v. Causal
masking multiplies the 4 distinct diagonal-block patterns post-exp; fully
masked blocks are skipped entirely.
"""

from contextlib import ExitStack

import numpy as np

import concourse.mybir as mybir
import concourse.tile as tile
from concourse import bacc
from concourse.bass import ts
from concourse.bass_utils import run_bass_kernel_spmd

B, S, D, H, HD = 4, 2048, 1024, 16, 64
P = 128
NHL = 8        # heads per core
DL = NHL * HD  # 512 local dims per of q/k/v
NT = S // 512  # 4 token tiles
NKB = S // P   # 16 k blocks
NQB = S // 512 # 4 q blocks
F32 = mybir.dt.float32
F32R = mybir.dt.float32r
AF = mybir.ActivationFunctionType
MULT = mybir.AluOpType.mult

_CACHE = {}


def build_program():
    if "nc" in _CACHE:
        return _CACHE["nc"]
    nc = bacc.Bacc("TRN2", target_bir_lowering=False, debug=False)
    # All host-side layouts are partition-contiguous so each DMA needs few
    # descriptors (descriptor generation serializes on the issuing sequencer).
    xT = nc.dram_tensor("xT", [NT, P, 8, 512], F32R, kind="ExternalInput").ap()
    wqk = nc.dram_tensor("wqk", [P, 8, 2 * DL], F32R, kind="ExternalInput").ap()
    wv = nc.dram_tensor("wv", [P, 8, DL], F32R, kind="ExternalInput").ap()
    bqk = nc.dram_tensor("bqk", [P, 8], F32, kind="ExternalInput").ap()
    wp = nc.dram_tensor("wp", [P, 4, D], F32R, kind="ExternalInput").ap()
    outT = nc.dram_tensor("outT", [P, 8, S], F32, kind="ExternalOutput").ap()

    with tile.TileContext(nc) as tc:
        with ExitStack() as stack:
            const = stack.enter_context(tc.tile_pool(name="const", bufs=1))
            poolAB = stack.enter_context(tc.tile_pool(name="poolAB", bufs=1))
            psAll = stack.enter_context(
                tc.tile_pool(name="ps", bufs=2, space="PSUM")
            )

            # ---- constants
            # TRI [128,128]: keep iff q_j >= k_i (diagonal triangle)
            # TRIZ [128,256]: keep iff q_j >= k_i + 128 (zeros col 0..128, then
            # triangle) -- used when the natural segment would be 128 wide,
            # which hits the narrow-f32r-matmul penalty.
            tri = const.tile([P, P], F32, tag="tri")
            nc.gpsimd.memset(tri[:], 1.0)
            nc.gpsimd.affine_select(
                out=tri[:],
                in_=tri[:],
                compare_op=mybir.AluOpType.is_ge,
                fill=0.0,
                base=0,
                pattern=[[1, P]],
                channel_multiplier=-1,
            )
            triz = const.tile([P, 2 * P], F32, tag="triz")
            nc.gpsimd.memset(triz[:], 1.0)
            nc.gpsimd.affine_select(
                out=triz[:],
                in_=triz[:],
                compare_op=mybir.AluOpType.is_ge,
                fill=0.0,
                base=-P,
                pattern=[[1, 2 * P]],
                channel_multiplier=-1,
            )
            bqk_sb = const.tile([P, 8], F32, tag="bqk")
            nc.sync.dma_start(bqk_sb[:], bqk)
            ones_row_f = const.tile([1, HD], F32, tag="ones_f")
            nc.gpsimd.memset(ones_row_f[:], 1.0)
            ones_row = const.tile([1, HD], F32R, tag="ones_r")
            nc.vector.tensor_copy(ones_row[:], ones_row_f[:])
            ones_v = const.tile([P, NKB, NHL], F32, tag="ones_v")
            nc.gpsimd.memset(ones_v[:], 1.0)

            # ---- persistent intermediates (phases A+B)
            qkT = poolAB.tile([P, 8, S], F32R, tag="qkT")  # chunks: q0..q3 k0..k3
            v_sb = poolAB.tile([P, NKB, NHL, HD + 1], F32R, tag="v")
            nc.vector.tensor_copy(v_sb[:, :, :, HD], ones_v[:])

            # =============== phase A: qkv projections =================
            with ExitStack() as stA:
                poolA = stA.enter_context(tc.tile_pool(name="poolA", bufs=1))
                xpool = stA.enter_context(tc.tile_pool(name="xpool", bufs=2))
                wqk_sb = poolA.tile([P, 8, 2 * DL], F32R, tag="wqk")
                wv_sb = poolA.tile([P, 8, DL], F32R, tag="wv")
                # chunked weight loads so the first matmul starts after ~2us
                # (DMA engines are a shared serial resource in practice)
                xt0 = xpool.tile([P, 8, 512], F32R, tag="xt", name="xt")
                nc.sync.dma_start(xt0[:, 0:4], xT[0, :, 0:4])
                nc.sync.dma_start(wqk_sb[:, 0, :], wqk[:, 0, :])
                nc.sync.dma_start(xt0[:, 4:8], xT[0, :, 4:8])
                for kc in range(1, 8):
                    nc.sync.dma_start(wqk_sb[:, kc, :], wqk[:, kc, :])
                for kc in range(8):
                    nc.sync.dma_start(wv_sb[:, kc, :], wv[:, kc, :])

                for tt in range(NT):
                    if tt == 0:
                        xt = xt0
                    else:
                        xt = xpool.tile([P, 8, 512], F32R, tag="xt", name="xt")
                        nc.sync.dma_start(xt[:, 0:4], xT[tt, :, 0:4])
                        nc.sync.dma_start(xt[:, 4:8], xT[tt, :, 4:8])
                    # q^T / k^T chunks [128 dims, 512 tokens]
                    for j in range(8):
                        pq = psAll.tile([P, 512], F32, tag="po", name="pq")
                        for kc in range(8):
                            nc.tensor.matmul(
                                pq[:],
                                wqk_sb[:, kc, ts(j, P)],
                                xt[:, kc, :],
                                start=(kc == 0),
                                stop=(kc == 7),
                            )
                        nc.scalar.activation(
                            qkT[:, j, ts(tt, 512)],
                            pq[:],
                            AF.Identity,
                            bias=bqk_sb[:, j : j + 1],
                        )
                    # v token-major [128 tokens, 512 vdims]
                    for u in range(4):
                        pv = psAll.tile([P, 512], F32, tag="po", name="pv")
                        for kc in range(8):
                            nc.tensor.matmul(
                                pv[:],
                                xt[:, kc, ts(u, P)],
                                wv_sb[:, kc, :],
                                start=(kc == 0),
                                stop=(kc == 7),
                            )
                        nc.vector.tensor_copy(
                            v_sb[:, tt * 4 + u, :, 0:HD],
                            pv[:].rearrange("p (h d) -> p h d", d=HD),
                        )

            # =============== phases B + C pools =================
            poolBC = stack.enter_context(tc.tile_pool(name="poolBC", bufs=1))
            stB = stack.enter_context(ExitStack())
            workB = stB.enter_context(tc.tile_pool(name="workB", bufs=5))
            workD = stB.enter_context(tc.tile_pool(name="workD", bufs=2))
            workC = stack.enter_context(tc.tile_pool(name="workC", bufs=2))

            yT = poolBC.tile([P, 4, S], F32R, tag="yT")
            wp_sb = poolBC.tile([P, 4, D], F32R, tag="wp")
            nc.sync.dma_start(wp_sb[:], wp)

            # =============== phase B: attention =================
            # Per head, per 1024-wide q half: k-blocks outer, with one wide
            # exp per (head, k-block). Scores live transposed in PSUM as
            # [k-tokens, q]; causality restricts each block to q >= 128*kb,
            # with segment starts widened so every matmul free-dim is >= 256.
            QW = 1024  # q half width

            def emit_division(h, half, po):
                # normalize: y^T = po[0:64] * broadcast(1/po[64])
                off = (h % 2) * HD
                j = h // 2
                qbase = QW * half
                rec = workD.tile([1, QW], F32R, tag="rec", name="rec")
                with nc.allow_low_precision(reason="f32r is fp32 storage"):
                    nc.vector.reciprocal(rec[:], po[HD : HD + 1, :])
                pr = psAll.tile([HD, QW], F32, tag="po", name="pr")
                for s0 in range(0, QW, 512):
                    nc.tensor.matmul(
                        pr[:, s0 : s0 + 512],
                        ones_row[:],
                        rec[:, s0 : s0 + 512],
                        start=True,
                        stop=True,
                    )
                onum = workD.tile([HD, QW], F32, tag="onum", name="onum")
                nc.vector.tensor_copy(onum[:], po[0:HD, :])
                nc.vector.tensor_tensor(
                    yT[off : off + HD, j, qbase : qbase + QW], onum[:], pr[:], MULT
                )

            def mk_C_group(oc, half):
                # one output-projection column chunk over this q half
                def emit():
                    osb = workC.tile([P, QW], F32, tag="osb", name="osb")
                    for u, tt in enumerate((2 * half, 2 * half + 1)):
                        pp = psAll.tile([P, 512], F32, tag="po", name="pp")
                        for dc in range(4):
                            nc.tensor.matmul(
                                pp[:],
                                wp_sb[:, dc, ts(oc, P)],
                                yT[:, dc, ts(tt, 512)],
                                start=(dc == 0),
                                stop=(dc == 3),
                            )
                        nc.vector.tensor_copy(osb[:, ts(u, 512)], pp[:])
                    nc.sync.dma_start(
                        outT[:, oc, half * QW : (half + 1) * QW], osb[:]
                    )

                return emit

            pending_div = None
            pending_C = []
            for half in range(2):
                for h in range(NHL):
                    off = (h % 2) * HD
                    j = h // 2
                    qbase = QW * half
                    kb_max = 8 * (half + 1)
                    # last k-block contributing to psum segment A ([0, 512))
                    lastA = (qbase + 512 + 127) // P - 1
                    po = psAll.tile([HD + 1, QW], F32, tag="po", name="po")
                    # software-pipelined emission: mm1/exp(kb+1) is emitted
                    # before mm2(kb) so the in-order PE stream never stalls
                    # on the exp latency.
                    pend_mm2 = None
                    for kb in range(kb_max):
                        d = P * kb - qbase  # diagonal column within this half
                        if d < 0:
                            c0, mask, mcol = 0, None, 0
                        elif d % 512 == 384:
                            c0, mask, mcol = d - P, triz, d - P
                        else:
                            c0, mask, mcol = d, tri, d
                        segs = []
                        if c0 < 512:
                            segs.append((c0, 512, lastA))
                        segs.append((max(c0, 512), QW, kb_max - 1))

                        pscr = psAll.tile([P, QW], F32, tag="pscr", name="pscr")
                        for s0, s1, _ in segs:
                            nc.tensor.matmul(
                                pscr[:, s0:s1],
                                qkT[off : off + HD, 4 + j, ts(kb, P)],
                                qkT[off : off + HD, j, qbase + s0 : qbase + s1],
                                start=True,
                                stop=True,
                            )
                        p_sb = workB.tile([P, QW], F32R, tag="p", name="p")
                        nc.scalar.activation(
                            p_sb[:, c0:QW], pscr[:, c0:QW], AF.Exp, scale=0.125
                        )
                        if mask is not None:
                            w = mask.shape[1]
                            nc.vector.tensor_tensor(
                                p_sb[:, mcol : mcol + w],
                                p_sb[:, mcol : mcol + w],
                                mask[:],
                                MULT,
                            )
                        if pend_mm2 is not None:
                            pend_mm2()
                        if pending_div is not None:
                            pending_div()
                            pending_div = None

                        def mk_mm2(segs=segs, p_sb=p_sb, kb=kb, h=h, po=po):
                            def emit():
                                for s0, s1, last in segs:
                                    nc.tensor.matmul(
                                        po[:, s0:s1],
                                        v_sb[:, kb, h, :],
                                        p_sb[:, s0:s1],
                                        start=(kb == 0),
                                        stop=(kb == last),
                                    )

                            return emit

                        pend_mm2 = mk_mm2()
                    pend_mm2()
                    pending_div = (
                        lambda h=h, half=half, po=po: emit_division(h, half, po)
                    )
                    # interleave one projection group of the previous half
                    # between attention heads (fills PE gaps without
                    # touching ACT or the pscr slots)
                    if pending_C:
                        pending_C.pop(0)()
                # ---- queue phase C for this q half (needs all heads' yT
                # for these tokens, complete once the half is done)
                pending_div()
                pending_div = None
                pending_C.extend(mk_C_group(oc, half) for oc in range(8))
            for emit_c in pending_C:
                emit_c()

    nc.compile()
    _CACHE["nc"] = nc
    return nc


def make_in_maps(x, W_attn, b_attn, W_proj):
    x = np.asarray(x, dtype=np.float32)
    W_attn = np.asarray(W_attn, dtype=np.float32)
    b_attn = np.asarray(b_attn, dtype=np.float32)
    W_proj = np.asarray(W_proj, dtype=np.float32)
    in_maps = []
    for c in range(8):
        b, g = divmod(c, 2)
        q_sl = slice(g * DL, (g + 1) * DL)
        k_sl = slice(D + g * DL, D + (g + 1) * DL)
        v_sl = slice(2 * D + g * DL, 2 * D + (g + 1) * DL)
        xTb = x[b].T  # [D, S]
        wqk = np.concatenate([W_attn[:, q_sl], W_attn[:, k_sl]], axis=1)
        in_maps.append(
            {
                "xT": np.ascontiguousarray(
                    xTb.reshape(8, P, NT, 512).transpose(2, 1, 0, 3)
                ),
                "wqk": np.ascontiguousarray(
                    wqk.reshape(8, P, 2 * DL).transpose(1, 0, 2)
                ),
                "wv": np.ascontiguousarray(
                    W_attn[:, v_sl].reshape(8, P, DL).transpose(1, 0, 2)
                ),
                "bqk": np.ascontiguousarray(
                    np.concatenate([b_attn[q_sl], b_attn[k_sl]]).reshape(8, P).T
                ),
                "wp": np.ascontiguousarray(
                    W_proj[g * DL : (g + 1) * DL, :].reshape(4, P, D).transpose(1, 0, 2)
                ),
            }
        )
    return in_maps


def combine(results, b_attn, W_proj, b_proj):
    const_row = b_attn[2 * D :].astype(np.float32) @ W_proj + b_proj
    out = np.empty((B, S, D), dtype=np.float32)
    for b in range(B):
        acc = results[2 * b]["outT"] + results[2 * b + 1]["outT"]  # [128, 8, S]
        out[b] = acc.transpose(1, 0, 2).reshape(D, S).T + const_row[None, :]
    return out


def kernel(x, W_attn, b_attn, W_proj, b_proj):
    nc = build_program()
    in_maps = make_in_maps(x, W_attn, b_attn, W_proj)
    res = run_bass_kernel_spmd(nc, in_maps, core_ids=list(range(8)))
    return combine(
        res.results,
        np.asarray(b_attn, np.float32),
        np.asarray(W_proj, np.float32),
        np.asarray(b_proj, np.float32),
    )


# revision 34
# speedup vs baseline: 15317.8587x; 15274.3606x over previous
"""Causal self-attention (B=4, S=2048, D=1024, H=16) on 8 Trainium2 NeuronCores.

Sharding: core c handles batch c//2 and heads (c%2)*8 .. (c%2)*8+8
(tensor-parallel over heads x data-parallel over batch). Each core:
  qkv-proj for its 1536 W_attn columns, attention for its 8 heads, and a
  row-parallel output projection producing a partial [2048, 1024]. The host
  sums the two partials per batch and adds the bias constant
  (b_v @ W_proj + b_proj -- valid because softmax rows sum to 1, so the
  v-bias passes through attention unchanged).

Device layouts: scores are computed transposed ([k-tokens on partitions,
q on free]) so the attention output lands as y^T [head-dims, tokens], which
feeds the output projection directly without any transposes. The softmax
denominator rides along as a 65th column of ones appended to v. Causal
masking multiplies the 4 distinct diagonal-block patterns post-exp; fully
masked blocks are skipped entirely.
"""

from contextlib import ExitStack

import numpy as np

import concourse.mybir as mybir
import concourse.tile as tile
from concourse import bacc
from concourse.bass import ts
from concourse.bass_utils import run_bass_kernel_spmd

B, S, D, H, HD = 4, 2048, 1024, 16, 64
P = 128
NHL = 8        # heads per core
DL = NHL * HD  # 512 local dims per of q/k/v
NT = S // 512  # 4 token tiles
NKB = S // P   # 16 k blocks
NQB = S // 512 # 4 q blocks
F32 = mybir.dt.float32
F32R = mybir.dt.float32r
AF = mybir.ActivationFunctionType
MULT = mybir.AluOpType.mult

_CACHE = {}


def build_program():
    if "nc" in _CACHE:
        return _CACHE["nc"]
    nc = bacc.Bacc("TRN2", target_bir_lowering=False, debug=False)
    # All host-side layouts are partition-contiguous so each DMA needs few
    # descriptors (descriptor generation serializes on the issuing sequencer).
    xT = nc.dram_tensor("xT", [NT, P, 8, 512], F32R, kind="ExternalInput").ap()
    wqk = nc.dram_tensor("wqk", [P, 8, 2 * DL], F32R, kind="ExternalInput").ap()
    wv = nc.dram_tensor("wv", [P, 8, DL], F32R, kind="ExternalInput").ap()
    bqk = nc.dram_tensor("bqk", [P, 8], F32, kind="ExternalInput").ap()
    wp = nc.dram_tensor("wp", [P, 4, D], F32R, kind="ExternalInput").ap()
    outT = nc.dram_tensor("outT", [P, 8, S], F32, kind="ExternalOutput").ap()

    with tile.TileContext(nc) as tc:
        with ExitStack() as stack:
            const = stack.enter_context(tc.tile_pool(name="const", bufs=1))
            poolAB = stack.enter_context(tc.tile_pool(name="poolAB", bufs=1))
            psAll = stack.enter_context(
                tc.tile_pool(name="ps", bufs=2, space="PSUM")
            )

            # ---- constants
            # TRI [128,128]: keep iff q_j >= k_i (diagonal triangle)
            # TRIZ [128,256]: keep iff q_j >= k_i + 128 (zeros col 0..128, then
            # triangle) -- used when the natural segment would be 128 wide,
            # which hits the narrow-f32r-matmul penalty.
            tri = const.tile([P, P], F32, tag="tri")
            nc.gpsimd.memset(tri[:], 1.0)
            nc.gpsimd.affine_select(
                out=tri[:],
                in_=tri[:],
                compare_op=mybir.AluOpType.is_ge,
                fill=0.0,
                base=0,
                pattern=[[1, P]],
                channel_multiplier=-1,
            )
            triz = const.tile([P, 2 * P], F32, tag="triz")
            nc.gpsimd.memset(triz[:], 1.0)
            nc.gpsimd.affine_select(
                out=triz[:],
                in_=triz[:],
                compare_op=mybir.AluOpType.is_ge,
                fill=0.0,
                base=-P,
                pattern=[[1, 2 * P]],
                channel_multiplier=-1,
            )
            bqk_sb = const.tile([P, 8], F32, tag="bqk")
            nc.sync.dma_start(bqk_sb[:], bqk)
            ones_row_f = const.tile([1, HD], F32, tag="ones_f")
            nc.gpsimd.memset(ones_row_f[:], 1.0)
            ones_row = const.tile([1, HD], F32R, tag="ones_r")
            nc.vector.tensor_copy(ones_row[:], ones_row_f[:])
            ones_v = const.tile([P, NKB, NHL], F32, tag="ones_v")
            nc.gpsimd.memset(ones_v[:], 1.0)

            # ---- persistent intermediates (phases A+B)
            qkT = poolAB.tile([P, 8, S], F32R, tag="qkT")  # chunks: q0..q3 k0..k3
            v_sb = poolAB.tile([P, NKB, NHL, HD + 1], F32R, tag="v")
            nc.vector.tensor_copy(v_sb[:, :, :, HD], ones_v[:])

            # =============== phase A: qkv projections =================
            with ExitStack() as stA:
                poolA = stA.enter_context(tc.tile_pool(name="poolA", bufs=1))
                xpool = stA.enter_context(tc.tile_pool(name="xpool", bufs=2))
                wqk_sb = poolA.tile([P, 8, 2 * DL], F32R, tag="wqk")
                wv_sb = poolA.tile([P, 8, DL], F32R, tag="wv")
                # chunked weight loads so the first matmul starts after ~2us
                # (DMA engines are a shared serial resource in practice)
                xt0 = xpool.tile([P, 8, 512], F32R, tag="xt", name="xt")
                for kc in range(8):
                    nc.sync.dma_start(xt0[:, kc : kc + 1], xT[0, :, kc : kc + 1])
                    nc.sync.dma_start(wqk_sb[:, kc, :], wqk[:, kc, :])
                for kc in range(8):
                    nc.sync.dma_start(wv_sb[:, kc, :], wv[:, kc, :])

                for tt in range(NT):
                    if tt == 0:
                        xt = xt0
                    else:
                        xt = xpool.tile([P, 8, 512], F32R, tag="xt", name="xt")
                        nc.sync.dma_start(xt[:, 0:4], xT[tt, :, 0:4])
                        nc.sync.dma_start(xt[:, 4:8], xT[tt, :, 4:8])
                    # q^T / k^T chunks [128 dims, 512 tokens]
                    for j in range(8):
                        pq = psAll.tile([P, 512], F32, tag="po", name="pq")
                        for kc in range(8):
                            nc.tensor.matmul(
                                pq[:],
                                wqk_sb[:, kc, ts(j, P)],
                                xt[:, kc, :],
                                start=(kc == 0),
                                stop=(kc == 7),
                            )
                        nc.scalar.activation(
                            qkT[:, j, ts(tt, 512)],
                            pq[:],
                            AF.Identity,
                            bias=bqk_sb[:, j : j + 1],
                        )
                    # v token-major [128 tokens, 512 vdims]
                    for u in range(4):
                        pv = psAll.tile([P, 512], F32, tag="po", name="pv")
                        for kc in range(8):
                            nc.tensor.matmul(
                                pv[:],
                                xt[:, kc, ts(u, P)],
                                wv_sb[:, kc, :],
                                start=(kc == 0),
                                stop=(kc == 7),
                            )
                        nc.vector.tensor_copy(
                            v_sb[:, tt * 4 + u, :, 0:HD],
                            pv[:].rearrange("p (h d) -> p h d", d=HD),
                        )

            # =============== phases B + C pools =================
            poolBC = stack.enter_context(tc.tile_pool(name="poolBC", bufs=1))
            stB = stack.enter_context(ExitStack())
            workB = stB.enter_context(tc.tile_pool(name="workB", bufs=5))
            workD = stB.enter_context(tc.tile_pool(name="workD", bufs=2))
            workC = stack.enter_context(tc.tile_pool(name="workC", bufs=2))

            yT = poolBC.tile([P, 4, S], F32R, tag="yT")
            wp_sb = poolBC.tile([P, 4, D], F32R, tag="wp")
            nc.sync.dma_start(wp_sb[:], wp)

            # =============== phase B: attention =================
            # Per head, per 1024-wide q half: k-blocks outer, with one wide
            # exp per (head, k-block). Scores live transposed in PSUM as
            # [k-tokens, q]; causality restricts each block to q >= 128*kb,
            # with segment starts widened so every matmul free-dim is >= 256.
            QW = 1024  # q half width

            def emit_division(h, half, po):
                # normalize: y^T = po[0:64] * broadcast(1/po[64])
                off = (h % 2) * HD
                j = h // 2
                qbase = QW * half
                rec = workD.tile([1, QW], F32R, tag="rec", name="rec")
                with nc.allow_low_precision(reason="f32r is fp32 storage"):
                    nc.vector.reciprocal(rec[:], po[HD : HD + 1, :])
                pr = psAll.tile([HD, QW], F32, tag="po", name="pr")
                for s0 in range(0, QW, 512):
                    nc.tensor.matmul(
                        pr[:, s0 : s0 + 512],
                        ones_row[:],
                        rec[:, s0 : s0 + 512],
                        start=True,
                        stop=True,
                    )
                onum = workD.tile([HD, QW], F32, tag="onum", name="onum")
                nc.vector.tensor_copy(onum[:], po[0:HD, :])
                nc.vector.tensor_tensor(
                    yT[off : off + HD, j, qbase : qbase + QW], onum[:], pr[:], MULT
                )

            def mk_C_group(oc, half):
                # one output-projection column chunk over this q half
                def emit():
                    osb = workC.tile([P, QW], F32, tag="osb", name="osb")
                    for u, tt in enumerate((2 * half, 2 * half + 1)):
                        pp = psAll.tile([P, 512], F32, tag="po", name="pp")
                        for dc in range(4):
                            nc.tensor.matmul(
                                pp[:],
                                wp_sb[:, dc, ts(oc, P)],
                                yT[:, dc, ts(tt, 512)],
                                start=(dc == 0),
                                stop=(dc == 3),
                            )
                        nc.vector.tensor_copy(osb[:, ts(u, 512)], pp[:])
                    nc.sync.dma_start(
                        outT[:, oc, half * QW : (half + 1) * QW], osb[:]
                    )

                return emit

            pending_div = None
            pending_C = []
            for half in range(2):
                for h in range(NHL):
                    off = (h % 2) * HD
                    j = h // 2
                    qbase = QW * half
                    kb_max = 8 * (half + 1)
                    # last k-block contributing to psum segment A ([0, 512))
                    lastA = (qbase + 512 + 127) // P - 1
                    po = psAll.tile([HD + 1, QW], F32, tag="po", name="po")
                    # software-pipelined emission: mm1/exp(kb+1) is emitted
                    # before mm2(kb) so the in-order PE stream never stalls
                    # on the exp latency.
                    pend_mm2 = None
                    for kb in range(kb_max):
                        d = P * kb - qbase  # diagonal column within this half
                        if d < 0:
                            c0, mask, mcol = 0, None, 0
                        elif d % 512 == 384:
                            c0, mask, mcol = d - P, triz, d - P
                        else:
                            c0, mask, mcol = d, tri, d
                        segs = []
                        if c0 < 512:
                            segs.append((c0, 512, lastA))
                        segs.append((max(c0, 512), QW, kb_max - 1))

                        pscr = psAll.tile([P, QW], F32, tag="pscr", name="pscr")
                        for s0, s1, _ in segs:
                            nc.tensor.matmul(
                                pscr[:, s0:s1],
                                qkT[off : off + HD, 4 + j, ts(kb, P)],
                                qkT[off : off + HD, j, qbase + s0 : qbase + s1],
                                start=True,
                                stop=True,
                            )
                        p_sb = workB.tile([P, QW], F32R, tag="p", name="p")
                        nc.scalar.activation(
                            p_sb[:, c0:QW], pscr[:, c0:QW], AF.Exp, scale=0.125
                        )
                        if mask is not None:
                            w = mask.shape[1]
                            nc.vector.tensor_tensor(
                                p_sb[:, mcol : mcol + w],
                                p_sb[:, mcol : mcol + w],
                                mask[:],
                                MULT,
                            )
                        if pend_mm2 is not None:
                            pend_mm2()
                        if pending_div is not None:
                            pending_div()
                            pending_div = None

                        def mk_mm2(segs=segs, p_sb=p_sb, kb=kb, h=h, po=po):
                            def emit():
                                for s0, s1, last in segs:
                                    nc.tensor.matmul(
                                        po[:, s0:s1],
                                        v_sb[:, kb, h, :],
                                        p_sb[:, s0:s1],
                                        start=(kb == 0),
                                        stop=(kb == last),
                                    )

                            return emit

                        pend_mm2 = mk_mm2()
                    pend_mm2()
                    pending_div = (
                        lambda h=h, half=half, po=po: emit_division(h, half, po)
                    )
                    # interleave one projection group of the previous half
                    # between attention heads (fills PE gaps without
                    # touching ACT or the pscr slots)
                    if pending_C:
                        pending_C.pop(0)()
                # ---- queue phase C for this q half (needs all heads' yT
                # for these tokens, complete once the half is done)
                pending_div()
                pending_div = None
                pending_C.extend(mk_C_group(oc, half) for oc in range(8))
            for emit_c in pending_C:
                emit_c()

    nc.compile()
    _CACHE["nc"] = nc
    return nc


def make_in_maps(x, W_attn, b_attn, W_proj):
    x = np.asarray(x, dtype=np.float32)
    W_attn = np.asarray(W_attn, dtype=np.float32)
    b_attn = np.asarray(b_attn, dtype=np.float32)
    W_proj = np.asarray(W_proj, dtype=np.float32)
    in_maps = []
    for c in range(8):
        b, g = divmod(c, 2)
        q_sl = slice(g * DL, (g + 1) * DL)
        k_sl = slice(D + g * DL, D + (g + 1) * DL)
        v_sl = slice(2 * D + g * DL, 2 * D + (g + 1) * DL)
        xTb = x[b].T  # [D, S]
        wqk = np.concatenate([W_attn[:, q_sl], W_attn[:, k_sl]], axis=1)
        in_maps.append(
            {
                "xT": np.ascontiguousarray(
                    xTb.reshape(8, P, NT, 512).transpose(2, 1, 0, 3)
                ),
                "wqk": np.ascontiguousarray(
                    wqk.reshape(8, P, 2 * DL).transpose(1, 0, 2)
                ),
                "wv": np.ascontiguousarray(
                    W_attn[:, v_sl].reshape(8, P, DL).transpose(1, 0, 2)
                ),
                "bqk": np.ascontiguousarray(
                    np.concatenate([b_attn[q_sl], b_attn[k_sl]]).reshape(8, P).T
                ),
                "wp": np.ascontiguousarray(
                    W_proj[g * DL : (g + 1) * DL, :].reshape(4, P, D).transpose(1, 0, 2)
                ),
            }
        )
    return in_maps


def combine(results, b_attn, W_proj, b_proj):
    const_row = b_attn[2 * D :].astype(np.float32) @ W_proj + b_proj
    out = np.empty((B, S, D), dtype=np.float32)
    for b in range(B):
        acc = results[2 * b]["outT"] + results[2 * b + 1]["outT"]  # [128, 8, S]
        out[b] = acc.transpose(1, 0, 2).reshape(D, S).T + const_row[None, :]
    return out


def kernel(x, W_attn, b_attn, W_proj, b_proj):
    nc = build_program()
    in_maps = make_in_maps(x, W_attn, b_attn, W_proj)
    res = run_bass_kernel_spmd(nc, in_maps, core_ids=list(range(8)))
    return combine(
        res.results,
        np.asarray(b_attn, np.float32),
        np.asarray(W_proj, np.float32),
        np.asarray(b_proj, np.float32),
    )
